# revision 31
# baseline (speedup 1.0000x reference)
"""Trainium2 Bass kernel for a 2-layer RGCN scene-graph model (8 NeuronCores).

v2 design:
- Featurize is graph-independent, so it is REPLICATED: every core computes
  x0 = [box|onehot(lab)] @ W for ALL 8192 nodes into a local fp8 gather table.
  This removes the first AllGather entirely. Block order is rotated per core
  (own blocks first) so the bf16 copy + transpose needed for the root term
  only covers the first 8 blocks -- the SPMD program stays identical and only
  the input data (boxT/labT column order, gather indices) is rotated.
- Layer 1 is dst-sharded exactly like the baseline: per (dst-block, relation)
  segment-mean via gathered-rows x one-hot matmuls, then the dense relation
  transform + root + bias, relu -> h1 (local 1024 nodes, fp8 rows to DRAM).
- Layer 2 is fused with the mean pooling (both linear): pooled[g] =
  sum_r Q_r[g] @ W2_r + mean_g(h1) @ RW2 + b2 with Q_r[g] =
  sum_{e in (r, dst in g)} h1[src_e] / (128*cnt). The (relation, graph)
  segment space is only 8x64, and the aggregation is SRC-sharded: each core
  aggregates the edges whose src it owns from its LOCAL h1 -- no second
  AllGather. Partial pooled sums [64, 256] are combined with a 64KB
  AllReduce; root/bias are scattered into global graph rows with a tiny
  per-core permutation matmul before the AllReduce. The classifier runs
  replicated on all cores; core 0's output is used.
- One-hot operands are built ON-CHIP by the idle DVE from per-slot
  (column, weight) tables via iota + fused is_equal/mult tensor_scalar,
  instead of DMA-ing ~13MB of mostly-zero one-hot matrices from HBM.
"""

import sys

sys.path.insert(0, "/opt/trn_rl_repo")

import numpy as np
import ml_dtypes

bf16 = ml_dtypes.bfloat16
fp8 = ml_dtypes.float8_e4m3
FP8 = True  # gather tables in float8_e4m3
FEAT_FP8 = False  # fp8 DoubleRow featurize measured slower on HW (A/B)

N = 8192
E = 262144
R = 8
NCLS = 151
EMB = 256
BOX = 1024
HID = 512
OUT = 256
NT = 2
CORES = 8
NLOC = N // CORES          # 1024 nodes per core
NB = NLOC // 128           # 8 dst blocks of 128 per core
NBT = N // 128             # 64 blocks total
GPC = NLOC // 128          # graphs per core
G = N // 128               # graphs total (nodes_per_graph == 128)

_PATCHED = False
DBG_SUB = 0  # 0=full layer, 1=gathers only, 2=+seg matmuls


def _patch_tile():
    """This container's walrus rejects >2 sync-wait commands per instruction;
    TileContext's kernel-tail drain attaches one wait per active logical proc.
    Redistribute the drain's waits over event-sem instructions (2 each)."""
    global _PATCHED
    if _PATCHED:
        return
    import concourse.mybir as mybir
    import concourse.tile as tile
    from concourse.vector_clock import ScopedClock

    def _drain_and_barrier(self, tick_clock, wait_clock):
        nc = self.nc
        drain_inst = nc.sync.drain()
        wait_clock.add_sem_waits(
            drain_inst.ins, ScopedClock({None: tick_clock.global_clock})
        )
        si = drain_inst.ins.sync_info
        waits = list(si.on_wait) if si is not None else []
        if waits:
            drain_inst.ins.sync_info = mybir.SyncInfo(
                on_wait=[], on_update=list(si.on_update) if si else []
            )
            dummy_sem = nc.alloc_semaphore(f"tail_split_sem_{nc.next_id()}")
            for i in range(0, len(waits), 2):
                ev = nc.sync.wait_ge(dummy_sem, 0)
                evsi = ev.ins.sync_info
                ev.ins.sync_info = mybir.SyncInfo(
                    on_wait=waits[i : i + 2],
                    on_update=list(evsi.on_update) if evsi else [],
                )
        nc.all_engine_barrier()
        assert self.sems is not None
        popped = nc._tile_sem_poison_stack.pop()
        assert popped is self._sem_poison
        nc.clear_and_free_semaphores(list(self.sems.allocated().values()))
        nc.all_engine_barrier()

    tile.TileContext._drain_and_barrier = _drain_and_barrier
    _PATCHED = True


def _split_excess_waits(nc, max_waits=2):
    """This walrus build rejects instructions carrying more than 2 sync-wait
    commands, but Tile's wait-assignment pass can attach more. Move excess
    waits onto same-engine EventSemaphore instructions inserted just before
    the over-subscribed instruction."""
    import concourse.mybir as mybir

    counter = [0]
    for f in nc.m.functions:
        for bb in f.blocks:
            cur = list(bb.instructions)
            out = []
            changed = False
            for ins in cur:
                si = ins.sync_info
                waits = list(si.on_wait) if si is not None else []
                allowed = (
                    max_waits
                    if type(ins).__name__ == "InstEventSemaphore"
                    else 1
                )
                if len(waits) > allowed:
                    keep = waits[:allowed]
                    extra = waits[allowed:]
                    ins.sync_info = mybir.SyncInfo(
                        on_wait=keep, on_update=list(si.on_update)
                    )
                    for i in range(0, len(extra), max_waits):
                        counter[0] += 1
                        ev = mybir.InstEventSemaphore(
                            name=f"I-wsplit-{counter[0]}",
                            ins=[],
                            outs=[],
                            engine=ins.engine,
                        )
                        ev.sync_info = mybir.SyncInfo(
                            on_wait=extra[i : i + max_waits], on_update=[]
                        )
                        out.append(ev)
                    changed = True
                out.append(ins)
            if changed:
                bb.instructions = out


def _wrap_idx(gidx):
    """[tot] int16 -> [128, tot//16] wrapped: column t holds slots
    16t..16t+15 on partitions 0-15, replicated to all 8 groups of 16."""
    tot = gidx.shape[0]
    w = gidx.reshape(tot // 16, 16).T  # [16, tot//16]
    out = np.zeros((128, tot // 16), dtype=np.int16)
    for rep in range(8):
        out[rep * 16 : (rep + 1) * 16, :] = w
    return out


def prep(inputs):
    """Host preprocessing: returns (in_maps, meta)."""
    box = np.asarray(inputs["box_features"], dtype=np.float32)
    lab = np.asarray(inputs["pred_labels"]).astype(np.int64).reshape(-1)
    ei = np.asarray(inputs["edge_index"]).astype(np.int64)
    et = np.asarray(inputs["edge_type"]).astype(np.int64).reshape(-1)
    emb = np.asarray(inputs["emb_table"], dtype=np.float32)
    W_lin = np.asarray(inputs["W_lin"], dtype=np.float32)
    b_lin = np.asarray(inputs["b_lin"], dtype=np.float32)
    rel_W1 = np.asarray(inputs["rel_W1"], dtype=np.float32)
    root_W1 = np.asarray(inputs["root_W1"], dtype=np.float32)
    b1 = np.asarray(inputs["b1"], dtype=np.float32)
    rel_W2 = np.asarray(inputs["rel_W2"], dtype=np.float32)
    root_W2 = np.asarray(inputs["root_W2"], dtype=np.float32)
    b2 = np.asarray(inputs["b2"], dtype=np.float32)
    Wc1 = np.asarray(inputs["Wc1"], dtype=np.float32)
    bc1 = np.asarray(inputs["bc1"], dtype=np.float32)
    Wc2 = np.asarray(inputs["Wc2"], dtype=np.float32)
    bc2 = np.asarray(inputs["bc2"], dtype=np.float32)

    src, dst = ei[0], ei[1]

    # per-(relation, dst) in-degree -> 1/cnt
    cnt = np.bincount(et * N + dst, minlength=R * N).reshape(R, N)
    inv = (1.0 / np.maximum(cnt, 1)).astype(np.float32)

    # ---------------- layer 1: dst-sharded (baseline scheme) ----------------
    core_of = dst // NLOC
    blk = (dst % NLOC) // 128

    key = (core_of * NB + blk) * R + et
    ecnt = np.bincount(key, minlength=CORES * NB * R).reshape(CORES, NB, R)
    chunks1 = np.ceil(ecnt / 128).astype(np.int64).max(axis=0)  # [NB, R]
    totc1 = int(chunks1.sum())
    tot1 = totc1 * 128

    goff1 = np.zeros((NB, R), dtype=np.int64)
    acc = 0
    for b in range(NB):
        for r in range(R):
            goff1[b, r] = acc
            acc += int(chunks1[b, r]) * 128

    # Per-core slot assignment. Within each (block, relation) group the edges
    # are sorted by ROTATED src row so that early chunks only reference a
    # bounded prefix of the xg table -- the per-quarter row bound (qbound)
    # lets layer-1 gathers start while featurize is still writing later
    # blocks.
    gidx1 = np.zeros((CORES, tot1), dtype=np.int16)
    dcol1 = np.zeros((CORES, tot1), dtype=np.float32)
    winv1 = np.zeros((CORES, tot1), dtype=np.float32)
    NPC = 4  # gather pieces per block (must match the device loop)
    qmax = np.zeros((CORES, NB, NPC), dtype=np.int64)
    for c in range(CORES):
        m = core_of == c
        c_src = src[m]
        c_dst = dst[m]
        c_et = et[m]
        c_blk = blk[m]
        rot = ((c_src // 128 - c * NB) % NBT) * 128 + c_src % 128
        order = np.lexsort((rot, c_et, c_blk))
        r_rot = rot[order]
        r_dst = c_dst[order]
        r_et = c_et[order]
        r_blk = c_blk[order]
        nE = r_rot.shape[0]
        grp_key = r_blk * R + r_et
        new_grp = np.empty(nE, dtype=bool)
        new_grp[0] = True
        new_grp[1:] = grp_key[1:] != grp_key[:-1]
        grp_start = np.flatnonzero(new_grp)
        start_of = np.repeat(grp_start, np.diff(np.append(grp_start, nE)))
        rank = np.arange(nE) - start_of
        slot = goff1[r_blk, r_et] + rank
        gidx1[c, slot] = r_rot.astype(np.int16)
        dcol1[c, slot] = (r_dst % 128).astype(np.float32)
        winv1[c, slot] = inv[r_et, r_dst]
        # per-(block, quarter) max referenced xg row
        g1 = gidx1[c].reshape(totc1, 128)
        coffq = 0
        for b in range(NB):
            nch = sum(int(chunks1[b][r]) for r in range(R))
            bounds = [nch * i // NPC for i in range(NPC + 1)]
            for q in range(NPC):
                c0, c1 = bounds[q], bounds[q + 1]
                if c1 > c0:
                    qmax[c, b, q] = int(g1[coffq + c0 : coffq + c1].max())
            coffq += nch
    # program-constant row bounds: max over cores, rounded up to a block
    qbound = qmax.max(axis=0)  # [NB, NPC]
    qbound = ((qbound // 128) + 1) * 128

    # [128, totc1] slot (chunk*128+p) -> [p, chunk]
    dcol1_t = dcol1.reshape(CORES, totc1, 128).transpose(0, 2, 1)
    winv1_t = winv1.reshape(CORES, totc1, 128).transpose(0, 2, 1)

    # ------------- layer 2: src-sharded, pooling-fused ----------------
    # Each core aggregates the edges whose src it owns, reading h1 straight
    # from SBUF (h1N). The per-src-block "one-hot" is a dense summed-weight
    # matrix: W2oh[p, sb, r, g] = sum over edges (r, src=sb*128+p, dst in g)
    # of inv[r,dst]/128.
    src_core = src // NLOC
    w2oh = np.zeros((CORES, 128, NB, R, G), dtype=np.float32)
    np.add.at(
        w2oh,
        (src_core, src % 128, (src % NLOC) // 128, et, dst // 128),
        inv[et, dst] / 128.0,
    )

    # ---------------- weights ----------------
    W_A = W_lin[:BOX]                                 # [1024, 512]
    W_Bc = emb @ W_lin[BOX:]                          # [151, 512]
    W_B = np.zeros((256, HID), dtype=np.float32)
    W_B[:NCLS] = W_Bc
    W_B[NCLS] = b_lin                                 # bias as a weight row

    def chunk_rows(Wm, p=128):
        K, O = Wm.shape
        return Wm.reshape(K // p, p, O).transpose(1, 0, 2).reshape(p, -1)

    W1_t = np.concatenate([chunk_rows(rel_W1[r]) for r in range(R)], axis=1)
    RW1_t = chunk_rows(root_W1)
    # the classifier is linear: fold Wc1 @ Wc2 into the layer-2 weights so the
    # cross-core reduction is over [64, 2] logits instead of [64, 256]
    Wc = Wc1 @ Wc2                                    # [256, 2]
    W2_t = np.concatenate(
        [chunk_rows(rel_W2[r] @ Wc) for r in range(R)], axis=1
    )                                                 # [128, R*4*2]
    RW2_t = chunk_rows(root_W2 @ Wc)                  # [128, 4*2]
    b2c = b2 @ Wc + bc1 @ Wc2 + bc2                   # [2]

    # full featurize operands, block-rotated per core (own blocks first)
    boxT = box.T.astype(np.float32)                   # [1024, 8192]
    boxT_k = boxT.reshape(8, 128, NBT, 128)           # [k, p, blk, n]
    # label-embedding contribution gathered on the host: added on-chip by the
    # DVE during the psum->SBUF copy, so the one-hot matmul chunks disappear
    embW_rows = (W_Bc[lab] + b_lin).astype(np.float32)  # [N, 512]
    embW_blk = embW_rows.reshape(NBT, 128, HID)

    W_AB8 = chunk_rows(W_A).astype(fp8 if FEAT_FP8 else bf16)

    shared = {

        "W1": W1_t.astype(bf16),
        "RW1": RW1_t.astype(bf16),
        "W2": W2_t.astype(bf16),
        "RW2": RW2_t.astype(bf16),
        "b1row": b1.reshape(1, HID).astype(bf16),
        "b2row": b2c.reshape(1, NT).astype(bf16),
        "ones1": np.ones((1, 128), dtype=bf16),
        "ones128": np.full((128, 1), 1.0 / 128.0, dtype=bf16),
        "iotaro": np.tile(np.arange(128, dtype=np.float32), (128, 1)).astype(bf16),
    }
    pack_order = list(shared.keys())

    in_maps = []
    offsets = None
    for c in range(CORES):
        rot_blocks = [(c * NB + i) % NBT for i in range(NBT)]
        m = dict(shared)
        blT8 = np.ascontiguousarray(
            boxT_k[:, :, rot_blocks, :].transpose(1, 2, 0, 3)
        ).reshape(128, 8 * N).astype(fp8 if FEAT_FP8 else bf16)
        m["embW"] = np.ascontiguousarray(
            embW_blk[rot_blocks].transpose(1, 0, 2)
        ).reshape(128, NBT * HID).astype(bf16)
        m["dcol1"] = dcol1_t[c].astype(bf16)
        m["winv1"] = winv1_t[c].astype(bf16)
        m["w2oh"] = w2oh[c].reshape(128, NB * R * G).astype(bf16)
        pm = np.zeros((8, G), dtype=np.float32)
        pm[np.arange(8), c * 8 + np.arange(8)] = 1.0
        m["Pmat"] = pm.astype(bf16)

        parts = pack_order + ["embW", "dcol1", "winv1", "w2oh", "Pmat"]
        offs = {}
        cur = 0
        bufs = []
        for name in parts:
            a = np.ascontiguousarray(m[name], dtype=bf16)
            offs[name] = (cur, a.shape)
            bufs.append(a.reshape(-1))
            cur += a.size
        blob = np.concatenate(bufs)
        if offsets is None:
            offsets = offs
        blob8 = np.concatenate(
            [blT8.reshape(-1), W_AB8.reshape(-1)]
        )
        in_maps.append(
            {
                "blob": blob.reshape(1, -1),
                "blob8": blob8.reshape(1, -1),
                "GIDX1": _wrap_idx(gidx1[c]),
            }
        )

    meta = {
        "qbound": tuple(tuple(int(x) for x in row) for row in qbound),
        "chunks1": tuple(tuple(int(x) for x in row) for row in chunks1),
        "feat_fp8": FEAT_FP8,
        "fp8": FP8,
        "offsets": offsets,
        "blob_elems": int(in_maps[0]["blob"].size),
        "blob8_elems": int(in_maps[0]["blob8"].size),
    }
    return in_maps, meta


def build(meta, split_waits=True, dbg_phase=99):
    _patch_tile()
    import concourse.bass as bass
    import concourse.mybir as mybir
    import concourse.tile as tile
    from concourse import library_config
    from concourse.bass import _add_dep_helper

    dt = mybir.dt
    AF = mybir.ActivationFunctionType
    ALU = mybir.AluOpType
    chunks1 = meta["chunks1"]
    qbound = meta["qbound"]
    totc1 = sum(sum(row) for row in chunks1)
    tot1 = totc1 * 128

    nc = bass.Bass()

    offsets = meta["offsets"]
    blob = nc.declare_dram_parameter(
        "blob", [1, meta["blob_elems"]], dt.bfloat16, isOutput=False
    )

    class _View:
        def __init__(self, ap):
            self._ap = ap

        def ap(self):
            return self._ap

        def __getitem__(self, idx):
            return self._ap[idx]

    class _ParamViews(dict):
        def __missing__(self, name):
            off, shape = offsets[name]
            p, c = shape
            ap = blob.ap()[:, off : off + p * c]
            ap = ap.rearrange("a (p c) -> (a p) c", p=p)
            v = _View(ap)
            self[name] = v
            return v

    P = _ParamViews()
    P["GIDX1"] = nc.declare_dram_parameter(
        "GIDX1", [128, tot1 // 16], dt.int16, isOutput=False
    )
    fdt = dt.float8e4 if meta.get("feat_fp8", True) else dt.bfloat16
    blob8 = nc.declare_dram_parameter(
        "blob8", [1, meta["blob8_elems"]], fdt, isOutput=False
    )
    out_y = nc.declare_dram_parameter("out", [2, G, NT], dt.float32, isOutput=True)

    # ---- internal DRAM ----
    # full x0 in bf16, rotated rows (own nodes are rows 0..NLOC-1)
    xg = nc.dram_tensor("xg", [N, HID], dt.bfloat16)
    y_part = nc.dram_tensor("y_part", [G, NT], dt.float32)
    y_all = nc.dram_tensor("y_all", [CORES * G, NT], dt.float32,
                           addr_space="Shared")

    rg = [list(range(CORES))]

    with tile.TileContext(nc) as tc:
        with (
            tc.tile_pool(name="wpool", bufs=1) as wpool,
            tc.tile_pool(name="xpool", bufs=1) as xpool,
            tc.tile_pool(name="spool", bufs=2) as spool,
            # single PSUM pool, 8 banks total:
            #   ps512 x2 (featurize + L1 segsum), transps x2 (L1 transform,
            #   reused by the classifier), qt0-3 x1 (L2 segsum, reused by the
            #   small tail tiles)
            tc.tile_pool(name="psP", bufs=1, space="PSUM") as psP,
        ):
            liblod = nc.gpsimd.load_library(library_config.mlp)

            def load(name, shape, dtype=dt.bfloat16, pool=wpool):
                t = pool.tile(list(shape), dtype, tag=name)
                nc.sync.dma_start(t[:], P[name].ap())
                return t

            # tensor_scalar is_equal requires float32 scalar operands
            def load_f32(name, cols):
                raw = wpool.tile([128, cols], dt.bfloat16, tag="scal_raw",
                                 name=name + "_raw", bufs=2)
                nc.sync.dma_start(raw[:], P[name].ap())
                f = wpool.tile(
                    [128, cols], dt.float32, tag=name + "_f", name=name + "_f"
                )
                nc.vector.tensor_copy(f[:], raw[:])
                return f


            # iota row tile: iota_bf[p, j] = j  (bf16, exact for 0..127)
            iota_bf = load("iotaro", (128, 128))

            # ---- replicated featurize: all 64 blocks, rotated order,
            # fp8 DoubleRow matmuls (2 k-chunks per instruction), groups of
            # 4 blocks per fused (box|lab) DMA ----
            FBG = 4
            bl8 = blob8.ap()[:, : 128 * 8 * N].rearrange(
                "a (p g kn) -> (a p) g kn", p=128, g=NBT // FBG
            )
            wab8 = blob8.ap()[:, 128 * 8 * N :].rearrange(
                "a (p kn) -> (a p) kn", p=128
            )
            embW_ap = P["embW"].ap().rearrange("p (nb f) -> p nb f", nb=NBT)
            fb_groups = [(g * FBG, FBG) for g in range(NBT // FBG)]
            fb_cm = tc.tile_pool(name="fbpool", bufs=3)
            fbpool = fb_cm.__enter__()
            W_AB_t = fbpool.tile([128, 8, HID], fdt, tag="W_AB",
                                 bufs=1)
            nc.sync.dma_start(
                W_AB_t[:].rearrange("p k f -> p (k f)"), wab8
            )
            bl8f = blob8.ap()[:, : 128 * 8 * N].rearrange(
                "a (p nb kn) -> (a p) nb kn", p=128, nb=NBT
            )
            for g0, gsz in fb_groups:
                bt = fbpool.tile([128, gsz, 8, 128], fdt, tag="bt")
                nc.sync.dma_start(
                    bt[:].rearrange("p a k n -> p (a k n)"),
                    bl8f[:, g0 : g0 + gsz, :].rearrange("p a kn -> p (a kn)"),
                )
                ew = fbpool.tile([128, gsz, HID], dt.bfloat16, tag="ew")
                nc.sync.dma_start(
                    ew[:].rearrange("p a f -> p (a f)"),
                    embW_ap[:, g0 : g0 + gsz, :].rearrange(
                        "p a f -> p (a f)"
                    ),
                )
                x0G = fbpool.tile([128, gsz, HID], dt.bfloat16, tag="x0G")
                for i in range(gsz):
                    nb = g0 + i
                    ps = psP.tile([128, HID], dt.float32, tag="ps512", bufs=2)
                    if meta.get("feat_fp8", True):
                        for k2 in range(4):
                            nc.tensor.matmul(
                                ps[:],
                                bt[:, i, 2 * k2 : 2 * k2 + 2, :],
                                W_AB_t[:, 2 * k2 : 2 * k2 + 2, :],
                                start=(k2 == 0), stop=(k2 == 3),
                                perf_mode=mybir.MatmulPerfMode.DoubleRow,
                            )
                    else:
                        for k in range(8):
                            nc.tensor.matmul(
                                ps[:], bt[:, i, k, :], W_AB_t[:, k, :],
                                start=(k == 0), stop=(k == 7),
                            )
                    # x0 = box-part (psum) + label-embedding rows, fused into
                    # the psum drain on the otherwise-idle DVE
                    nc.vector.tensor_tensor(
                        x0G[:, i, :], ps[:], ew[:, i, :], op=ALU.add
                    )
                nc.sync.dma_start(
                    xg.ap()[g0 * 128 : (g0 + gsz) * 128, :].rearrange(
                        "(a p) f -> p a f", p=128
                    ),
                    x0G[:],
                )
            fb_cm.__exit__(None, None, None)

            # bulk weight/index loads: issued after the featurize DMAs so
            # they queue behind them and don't delay the first blocks
            W1_t = load("W1", (128, R * 4 * HID))
            RW1_t = load("RW1", (128, 4 * HID))
            W2_t = load("W2", (128, R * 4 * NT))
            RW2_t = load("RW2", (128, 4 * NT))
            b1_t = load("b1row", (1, HID))
            b2_t = load("b2row", (1, NT))
            ones1_t = load("ones1", (1, 128))
            ones128_t = load("ones128", (128, 1))
            dcol1_t = load_f32("dcol1", totc1)
            winv1_t = load_f32("winv1", totc1)
            w2oh_t = load("w2oh", (128, NB * R * G))
            Pmat_t = load("Pmat", (8, G))
            GIDX1_t = load("GIDX1", (128, tot1 // 16), dt.int16)

            # transposed local x0 for the layer-1 root term
            x0T_t = xpool.tile([128, 4, NLOC], dt.bfloat16, tag="x0T")
            for k in range(4):
                nc.sync.dma_start(
                    x0T_t[:, k, :],
                    xg[:NLOC, k * 128 : (k + 1) * 128],
                    transpose=True,
                )

            def dummy_out():
                zz = spool.tile([G, NT], dt.float32, tag="ylog")
                nc.vector.memset(zz[:], 0.0)
                nc.sync.dma_start(out_y[0], zz[:])
                nc.sync.dma_start(out_y[1], zz[:])

            if dbg_phase < 3:
                dummy_out()
            else:
                _build_rest(
                    nc, tc, mybir, dt, AF, ALU, chunks1, P,
                    xg, x0T_t, y_part, y_all, rg, out_y,
                    xpool, spool, psP,
                    W1_t, RW1_t, W2_t, RW2_t, b1_t, b2_t,
                    ones1_t, ones128_t, Pmat_t,
                    dcol1_t, winv1_t, w2oh_t, iota_bf,
                    GIDX1_t, liblod, dbg_phase, qbound,
                )

    mybir.codegen_inst_isa_subclasses(nc)
    if split_waits:
        _split_excess_waits(nc)
    return nc


def _build_rest(nc, tc, mybir, dt, AF, ALU, chunks1, P,
                xg, x0T_t, y_part, y_all, rg, out_y,
                xpool, spool, psP,
                W1_t, RW1_t, W2_t, RW2_t, b1_t, b2_t,
                ones1_t, ones128_t, Pmat_t,
                dcol1_t, winv1_t, w2oh_t, iota_bf,
                GIDX1_t, liblod, dbg_phase, qbound):
    from concourse.bass import _add_dep_helper

    lp = tc.tile_pool(name="gpool", bufs=5)
    gpool = lp.__enter__()
    lp2 = tc.tile_pool(name="ohpool", bufs=5)
    ohpool = lp2.__enter__()
    lp3 = tc.tile_pool(name="stpool", bufs=2)
    stpool = lp3.__enter__()

    def build_oh(oh, ncols, coff, ncol_cols, col_t, w_t):
        """oh[:, j*ncol_cols:(j+1)*ncol_cols] =
        (iota == col_t[:, coff+j]) * w_t[:, coff+j] per chunk j."""
        for j in range(ncols):
            nc.vector.tensor_scalar(
                out=oh[:, j * ncol_cols : (j + 1) * ncol_cols],
                in0=iota_bf[:, :ncol_cols],
                scalar1=col_t[:, coff + j : coff + j + 1],
                scalar2=w_t[:, coff + j : coff + j + 1],
                op0=ALU.is_equal,
                op1=ALU.mult,
            )

    # ---------------- layer 1 (dst-sharded) ----------------
    h1N = xpool.tile([128, NB, HID], dt.bfloat16, tag="h1N")
    # layer-2 pooled aggregation psum: Q.T[f, (r, g)], one bank per f-chunk
    qt_ps = [
        psP.tile([128, R, 64], dt.float32, tag=f"qt{fc}", bufs=1,
                 name=f"qt{fc}")
        for fc in range(4)
    ]
    q_start = [None] * 4
    q_last = [None] * 4
    w2oh_v = w2oh_t[:].rearrange("p (sb r g) -> p sb r g", sb=NB, r=R)
    coff = 0
    for b in range(NB):
        chb = [chunks1[b][r] for r in range(R)]
        nch = sum(chb)
        if b == 0:
            # tiny first piece: minimizes the PE stall between the last
            # featurize matmul and the first segsum matmul (one gather deep)
            bounds = [0, 2, 6, nch // 2, nch * 3 // 4, nch]
        else:
            NPC = 4  # gather pieces per block
            bounds = [nch * i // NPC for i in range(NPC + 1)]
        pieces = []
        for q, (c0, c1) in enumerate(zip(bounds[:-1], bounds[1:])):
            ncols = c1 - c0
            gt = gpool.tile([128, max(ncols, 1), HID], dt.bfloat16, tag="gt1")
            if ncols > 0:
                g_ins = nc.gpsimd.dma_gather(
                    gt[:, :ncols, :],
                    xg.ap()[: qbound[b][min(q, len(qbound[b]) - 1)], :],
                    GIDX1_t[:, (coff + c0) * 8 : (coff + c1) * 8],
                    num_idxs=ncols * 128,
                    num_idxs_reg=ncols * 128,
                    elem_size=HID,
                    single_packet=False,
                )
                _add_dep_helper(
                    g_ins.ins, liblod.ins,
                    reason="dma_gather needs mlp library",
                )
            oh = ohpool.tile([128, max(ncols, 1) * 128], dt.bfloat16, tag="oh")
            if ncols > 0:
                build_oh(oh, ncols, coff + c0, 128, dcol1_t, winv1_t)
            pieces.append((gt, oh, c0, c1))
        # segment sums, transposed: S.T[f, dst] per relation
        st = []
        j0 = 0
        for r in range(R):
            nchr = chb[r]
            if nchr == 0 or DBG_SUB == 1:
                st.append(None)
                j0 += nchr
                continue
            pss = psP.tile([128, HID], dt.float32, tag="ps512", bufs=2)
            starter = None
            last_by_fc = {}
            for j in range(j0, j0 + nchr):
                for gt, oh, c0, c1 in pieces:
                    if c0 <= j < c1:
                        break
                jl = j - c0
                for fc in range(4):
                    mm = nc.tensor.matmul(
                        pss[:, fc * 128 : (fc + 1) * 128],
                        gt[:, jl, fc * 128 : (fc + 1) * 128],
                        oh[:, jl * 128 : (jl + 1) * 128],
                        start=(j == j0 and fc == 0),
                        stop=(j == j0 + nchr - 1 and fc == 3),
                    )
                    if starter is None:
                        starter = mm
                    elif j == j0:
                        _add_dep_helper(
                            mm.ins, starter.ins,
                            reason="psum group starter first",
                        )
                    if j == j0 + nchr - 1:
                        last_by_fc[fc] = mm
            for fc in range(3):
                _add_dep_helper(
                    last_by_fc[3].ins, last_by_fc[fc].ins,
                    reason="psum group stopper last",
                )
            stt = stpool.tile([128, HID], dt.bfloat16, tag=f"st{r}")
            nc.scalar.activation(stt[:], pss[:], AF.Copy)
            st.append(stt)
            j0 += nchr
        coff += nch
        if DBG_SUB in (1, 2):
            nc.vector.memset(h1N[:, b, :], 0.0)
            continue
        # transform: h1[dst, :] = relu(sum_r S_r.T.T @ W1_r + x0.T.T @ RW1 + b1)
        pst = psP.tile([128, HID], dt.float32, tag="transps", bufs=2)
        nc.tensor.matmul(
            pst[:], ones1_t[:1, :], b1_t[:1, :], start=True, stop=False
        )
        for r in range(R):
            if st[r] is None:
                continue
            for k in range(4):
                nc.tensor.matmul(
                    pst[:],
                    st[r][:, k * 128 : (k + 1) * 128],
                    W1_t[:, (r * 4 + k) * HID : (r * 4 + k + 1) * HID],
                    start=False, stop=False,
                )
        for k in range(4):
            nc.tensor.matmul(
                pst[:],
                x0T_t[:, k, b * 128 : (b + 1) * 128],
                RW1_t[:, k * HID : (k + 1) * HID],
                start=False, stop=(k == 3),
            )
        nc.scalar.activation(h1N[:, b, :], pst[:], AF.Relu)
        # layer-2 pooled aggregation for src-block b: h1 rows are already in
        # SBUF; the dense summed-weight matrix replaces gather + one-hot
        for r2 in range(R):
            for fc in range(4):
                mm = nc.tensor.matmul(
                    qt_ps[fc][:, r2, :],
                    h1N[:, b, fc * 128 : (fc + 1) * 128],
                    w2oh_v[:, b, r2, :],
                    start=(q_start[fc] is None),
                    stop=(b == NB - 1 and r2 == R - 1),
                )
                if q_start[fc] is None:
                    q_start[fc] = mm
                q_last[fc] = mm

    if dbg_phase < 4:
        lp3.__exit__(None, None, None)
        lp2.__exit__(None, None, None)
        lp.__exit__(None, None, None)
        zz = spool.tile([128, NT], dt.float32, tag="ylog_d")
        nc.vector.memset(zz[:], 0.0)
        nc.sync.dma_start(out_y[0], zz[:64, :])
        nc.sync.dma_start(out_y[1], zz[:64, :])
        return

    # qt accumulation happened inside the layer-1 loop
    qt_s = xpool.tile([128, 4, R, 64], dt.bfloat16, tag="qt_s")
    for fc in range(4):
        act = nc.scalar.activation(qt_s[:, fc], qt_ps[fc][:], AF.Copy)
        _add_dep_helper(act.ins, q_last[fc].ins, reason="qt read after stop")

    lp3.__exit__(None, None, None)
    lp2.__exit__(None, None, None)
    lp.__exit__(None, None, None)

    # pooled root input: hp[f, g_own] = mean over each own graph's 128 nodes
    # (emitted before the L2 segsum consumers in program order via deps only;
    # PE executes in order, but these depend only on h1N so they fill the
    # gather-wait gaps at the start of the L2 phase)
    hp_ps = psP.tile([128, 4, NB], dt.float32, tag="qt0", bufs=1)
    for b in range(NB):
        for fc in range(4):
            nc.tensor.matmul(
                hp_ps[:, fc, b : b + 1],
                h1N[:, b, fc * 128 : (fc + 1) * 128],
                ones128_t[:],
                start=(b == 0 and fc == 0),
                stop=(b == NB - 1 and fc == 3),
            )
    hp_s = spool.tile([128, 4, NB], dt.bfloat16, tag="hp_s")
    nc.scalar.activation(hp_s[:], hp_ps[:], AF.Copy)

    # root8 = hp.T @ (RW2 @ Wc) + folded bias   [8 own graphs, 2]
    root8_ps = psP.tile([8, NT], dt.float32, tag="qt1", bufs=1)
    nc.tensor.matmul(
        root8_ps[:], ones1_t[:1, :8], b2_t[:1, :], start=True, stop=False
    )
    for fc in range(4):
        nc.tensor.matmul(
            root8_ps[:],
            hp_s[:, fc, :],
            RW2_t[:, fc * NT : (fc + 1) * NT],
            start=False, stop=(fc == 3),
        )
    root8_s = spool.tile([8, NT], dt.bfloat16, tag="root8_s")
    nc.scalar.activation(root8_s[:], root8_ps[:], AF.Copy)

    # partial logits [64, 2]: sum_r Q_r.T.T @ (W2_r @ Wc) + scatter(root8)
    yp_ps = psP.tile([64, NT], dt.float32, tag="qt2", bufs=1)
    nc.tensor.matmul(
        yp_ps[:], Pmat_t[:, :], root8_s[:, :], start=True, stop=False
    )
    for r in range(R):
        for fc in range(4):
            nc.tensor.matmul(
                yp_ps[:],
                qt_s[:, fc, r, :],
                W2_t[:, (r * 4 + fc) * NT : (r * 4 + fc + 1) * NT],
                start=False, stop=(r == R - 1 and fc == 3),
            )
    ypart_s = spool.tile([64, NT], dt.float32, tag="ypart")
    nc.vector.tensor_copy(ypart_s[:], yp_ps[:])
    nc.sync.dma_start(y_part.ap(), ypart_s[:])

    # ---- AllGather partial logits (cheaper than AllReduce), sum on DVE ----
    nc.gpsimd.collective_compute(
        "AllGather",
        mybir.AluOpType.bypass,
        replica_groups=rg,
        ins=[y_part.ap().opt()],
        outs=[y_all.ap().opt()],
    )
    yall_s = spool.tile([64, NT, CORES], dt.float32, tag="yall")
    nc.sync.dma_start(
        yall_s[:], y_all.ap().rearrange("(c g) t -> g t c", c=CORES)
    )
    ylog = spool.tile([64, NT], dt.float32, tag="ylog")
    nc.vector.reduce_sum(
        ylog[:].rearrange("g (t one) -> g t one", one=1),
        yall_s[:],
        axis=mybir.AxisListType.X,
    )
    # softmax is shift-invariant and the logits here are O(0.1), so the
    # usual max-subtraction is skipped to shorten the post-collective chain
    ey = spool.tile([64, NT], dt.float32, tag="ey")
    nc.scalar.activation(ey[:], ylog[:], AF.Exp)
    ssum = spool.tile([64, 1], dt.float32, tag="ssum")
    nc.vector.reduce_sum(ssum[:], ey[:], axis=mybir.AxisListType.X)
    rinv = spool.tile([64, 1], dt.float32, tag="rinv")
    nc.vector.reciprocal(rinv[:], ssum[:])
    yprob = spool.tile([64, NT], dt.float32, tag="yprob")
    nc.vector.tensor_scalar_mul(yprob[:], ey[:], rinv[:])

    nc.sync.dma_start(out_y[0], ylog[:])
    nc.sync.dma_start(out_y[1], yprob[:])
